# revision 1
# baseline (speedup 1.0000x reference)
"""BitFeedForward (BitNet-style FFN) Trainium2 kernel — 8-core data parallel.

kernel(**inputs) takes the FULL unsharded inputs of
nn_BitFeedForward_25280177504455:
    x  [4, 2048, 2048] f32, w1 [8192, 2048], b1 [8192],
    w2 [2048, 8192], b2 [2048]
and returns the full [4, 2048, 2048] f32 output.

Sharding: pure data-parallel over tokens. x is flattened to [8192, 2048]
and split into 8 blocks of 1024 tokens; every core receives the full
weights plus its token block, runs an identical Bass program (no
collectives), and the host concatenates the 8 output blocks.

See build() for the on-device algorithm; the key algebraic facts are
  - activation_quant(rms_norm(x)) produces integers q in [-127, 127]
    with q = round(x * a*127/max(a*max|x|, eps)); ints are exact in bf16
  - weight_quant(w) = sign(w) * mean|w|, so each matmul is
    (q @ sign(w).T) * beta + b with beta = mean|w| * c / 127,
    computed exactly as a bf16 x bf16 matmul with fp32 PSUM accumulation
  - the bias rides each PSUM accumulation group as a rank-1 fp32 matmul
    (1/beta outer b), so beta fuses into the PSUM-reading ACT op.
"""
import functools

import numpy as np
import ml_dtypes

from contextlib import ExitStack

import concourse.bacc as bacc
import concourse.tile as tile
from concourse import mybir
from concourse.bass_utils import run_bass_kernel_spmd

F32 = mybir.dt.float32
BF16 = mybir.dt.bfloat16

EPS_RMS = 1e-6
EPS_Q = 1e-5
# v + C lands in [2^23, 2^24) where fp32 spacing is 1.0 -> RNE integer round
C_RND = float(1.5 * 2.0**23)
P = 128
AX = mybir.AxisListType
ALU = mybir.AluOpType
AF = mybir.ActivationFunctionType

NCORES = 8
B, S, DIM = 4, 2048, 2048
INNER = 8192
TOK = B * S // NCORES  # tokens per core


def build(TOK, D, INNER, OUT):
    from concourse.tile_rust import add_dep_helper

    TT = TOK // P
    KD = D // P
    KI = INNER // P
    NC1 = min(512, INNER)
    NCH = INNER // NC1
    NC2 = min(512, OUT)
    OCH = OUT // NC2
    WCC1 = min(1024, D)
    WCC2 = min(1024, INNER)
    SL = min(512, INNER)
    SL2 = min(512, OUT)
    nA = list(range(NCH // 2 if NCH > 1 else NCH))
    nB = list(range(len(nA), NCH))
    KIH = KI // 2
    G2W = min(2, KIH)
    halfW1 = [len(nA) * NC1, len(nB) * NC1]

    nc = bacc.Bacc("TRN2", enable_partition_id=False)

    x_d = nc.dram_tensor("x", [TOK, D], F32, kind="ExternalInput")
    w1_d = nc.dram_tensor("w1", [INNER, D], F32, kind="ExternalInput")
    b1_d = nc.dram_tensor("b1", [1, INNER], F32, kind="ExternalInput")
    w2_d = nc.dram_tensor("w2", [OUT, INNER], F32, kind="ExternalInput")
    b2_d = nc.dram_tensor("b2", [1, OUT], F32, kind="ExternalInput")
    idf_d = nc.dram_tensor("identf", [P, P], F32, kind="ExternalInput")
    idb_d = nc.dram_tensor("identb", [P, P], BF16, kind="ExternalInput")
    ones_d = nc.dram_tensor("ones", [P, P], F32, kind="ExternalInput")
    out_d = nc.dram_tensor("out", [TOK, OUT], F32, kind="ExternalOutput")

    with ExitStack() as ctx:
        tc = ctx.enter_context(tile.TileContext(nc))
        pool = lambda name, bufs, space="SBUF": ctx.enter_context(
            tc.tile_pool(name=name, bufs=bufs, space=space))

        consts = pool("consts", 1)
        bch = pool("bch", 2)
        dram = pool("dram", 1, "DRAM")
        wload = pool("wload", 2)
        wsign = pool("wsign", 2)
        junkp = pool("junk", 1)
        wT = pool("wT", 16)
        xp = pool("xp", 2)
        qp = pool("qp", 1)
        qTp = pool("qTp", 1)
        hcp = pool("hcp", 2)
        q2Tp = pool("q2Tp", 1)
        outp = pool("outp", 2)
        vecs = pool("vecs", 2)
        pers = pool("pers", 1)
        ps_g = pool("ps_g", 4, "PSUM")
        ps_t = pool("ps_t", 2, "PSUM")
        ps_v = pool("ps_v", 2, "PSUM")

        identf = consts.tile([P, P], F32)
        identb = consts.tile([P, P], BF16)
        ones = consts.tile([P, P], F32)
        nc.sync.dma_start(identf, idf_d[:, :])
        nc.sync.dma_start(identb, idb_d[:, :])
        nc.sync.dma_start(ones, ones_d[:, :])

        w1s_dram = dram.tile([KD, INNER, P], BF16)
        w2s_dram = dram.tile([KI, OUT, P], BF16)
        q2_dram = dram.tile([TOK, INNER], BF16)
        h_dram = dram.tile([TOK, INNER], F32)
        q1T_dram = dram.tile([TOK, D], BF16)
        opart_dram = dram.tile([TOK, OUT], F32)

        w1sums = pers.tile([P, (INNER // P) * (D // WCC1)], F32, tag="w1sums")
        w2sums = pers.tile([P, (OUT // P) * (INNER // WCC2)], F32,
                           tag="w2sums")

        state = {"pe": None}

        def pe(instr):
            if state["pe"] is not None:
                add_dep_helper(instr.ins, state["pe"].ins, sync=False,
                               reason="pe chain")
            state["pe"] = instr
            return instr

        wu1 = ps_v.tile([1, P], F32, tag="ps_v")
        pe(nc.tensor.transpose(wu1, identf[:, 0:1], identf))
        wu2 = ps_t.tile([P, P], BF16, tag="ps_qt")
        pe(nc.tensor.transpose(wu2, identb, identb))

        def w_prep(src_d, ROWS, COLS, CC, ws_dram, wsums, rows=None,
                   idx0=0, ldma=None):
            # ws_dram is [COLS//P, ROWS, P] (k-tiled); spill scatters each
            # sign chunk into per-k blocks (contiguous 256B rows) so the
            # transposed fill later reads contiguous megabyte slices.
            ldma = ldma or nc.sync
            idx = idx0
            r0, r1_ = rows if rows is not None else (0, ROWS)
            for i in range(r0 // P, r1_ // P):
                for cc in range(COLS // CC):
                    wt = wload.tile([P, CC], F32, tag="wload")
                    ldma.dma_start(
                        wt, src_d[i * P:(i + 1) * P, cc * CC:(cc + 1) * CC])
                    ws = wsign.tile([P, CC], BF16, tag="wsign")
                    nc.scalar.sign(ws, wt)
                    jk = junkp.tile([P, max(WCC1, WCC2)], BF16, tag="junk")
                    nc.scalar.activation(jk[:, :CC], wt, AF.Abs,
                                         accum_out=wsums[:, idx:idx + 1])
                    nc.gpsimd.dma_start(
                        out=ws_dram[cc * (CC // P):(cc + 1) * (CC // P),
                                    i * P:(i + 1) * P, :].rearrange(
                                        "k p c -> p k c"),
                        in_=ws.rearrange("p (k c) -> p k c", c=P))
                    idx += 1
            return idx

        def w_mean(wsums, nsum, nelem, tag):
            col = vecs.tile([P, 1], F32, tag="wm_col")
            nc.vector.reduce_sum(col, wsums[:, :nsum], axis=AX.X)
            pssc = ps_v.tile([1, 1], F32, tag="ps_v")
            pe(nc.tensor.matmul(pssc, col, ones[:, 0:1], start=True,
                                stop=True))
            sc = vecs.tile([1, 1], F32, tag="wm_sc")
            nc.scalar.copy(sc, pssc)
            psbc = ps_v.tile([P, 1], F32, tag="ps_v")
            pe(nc.tensor.matmul(psbc, ones[0:1, :], sc, start=True, stop=True))
            mw = pers.tile([P, 1], F32, tag=tag)
            nc.scalar.mul(mw, psbc, 1.0 / (nelem * 127.0))
            dmy = ps_v.tile([1, P], F32, tag="ps_v")
            pe(nc.tensor.transpose(dmy, mw, identf))
            return mw

        def w_fill(ws_dram, tiles, row0, rows, slice_len, kslice):
            for k in kslice:
                t = tiles[k % len(tiles)]
                nc.sync.dma_start_transpose(
                    t[:, 0:rows], ws_dram[k, row0:row0 + rows, :])

        def finalize_scale(stv, M, WID):
            mv = vecs.tile([P, 2], F32, tag="bn_mv")
            nc.vector.bn_aggr(mv, stv)
            msq = vecs.tile([P, 1], F32, tag="msq")
            nc.vector.tensor_tensor(msq, mv[:, 0:1], mv[:, 0:1], op=ALU.mult)
            nc.vector.tensor_tensor(msq, msq, mv[:, 1:2], op=ALU.add)
            nc.vector.tensor_scalar_add(msq, msq, EPS_RMS)
            y = vecs.tile([P, 1], F32, tag="sq_y")
            nc.scalar.sqrt(y, msq)
            d_ = vecs.tile([P, 1], F32, tag="sq_d")
            nc.vector.reciprocal(d_, y)
            nc.vector.tensor_tensor(d_, msq, d_, op=ALU.mult)
            nc.vector.tensor_tensor(y, y, d_, op=ALU.add)
            nc.vector.tensor_scalar_mul(y, y, 0.5 * (float(WID) ** 0.5))
            a = vecs.tile([P, 1], F32, tag="a")
            nc.vector.reciprocal(a, y)
            c = vecs.tile([P, 1], F32, tag="c")
            nc.vector.tensor_tensor(c, a, M, op=ALU.mult)
            nc.vector.tensor_scalar_max(c, c, EPS_Q)
            r = vecs.tile([P, 1], F32, tag="r")
            nc.vector.reciprocal(r, c)
            nc.vector.tensor_tensor(r, r, a, op=ALU.mult)
            nc.vector.tensor_scalar_mul(r, r, 127.0)
            return r, c

        def beta_of(c, mw, beta, rbT):
            nc.vector.tensor_tensor(beta, c, mw, op=ALU.mult)
            rb = vecs.tile([P, 1], F32, tag="rb")
            nc.vector.reciprocal(rb, beta)
            pst = ps_v.tile([1, P], F32, tag="ps_v")
            pe(nc.tensor.transpose(pst, rb, identf))
            nc.scalar.copy(rbT, pst)
            return beta, rbT

        def quant_chunk(src, q_out, r):
            nc.vector.tensor_scalar(src, src, r, C_RND, op0=ALU.mult,
                                    op1=ALU.add)
            nc.vector.tensor_scalar(q_out, src, C_RND, None,
                                    op0=ALU.subtract)

        def q_transpose(q, qT, kslice):
            pe(nc.tensor.ldweights(weights=q[:, kslice[0] * P:
                                            (kslice[0] + 1) * P]))
            for j, k in enumerate(kslice):
                pst = ps_t.tile([P, P], BF16, tag="ps_qt")
                pe(nc.tensor.transpose(pst, q[:, k * P:(k + 1) * P], identb))
                nc.scalar.copy(qT[:, j, :], pst)

        # ================= weight prep =================
        w_prep(w1_d, INNER, D, WCC1, w1s_dram, w1sums)
        mw1 = w_mean(w1sums, (INNER // P) * (D // WCC1), INNER * D, "mw1")
        w2_chunks_total = (OUT // P) * (INNER // WCC2)
        w2_state = {"idx": 0, "row": 0}

        def w2_prep_piece(nrows):
            r0 = w2_state["row"]
            r1_ = min(OUT, r0 + nrows)
            if r0 >= r1_:
                return
            w2_state["idx"] = w_prep(
                w2_d, OUT, INNER, WCC2, w2s_dram, w2sums,
                rows=(r0, r1_), idx0=w2_state["idx"], ldma=nc.scalar)
            w2_state["row"] = r1_

        w1T = [wT.tile([P, max(halfW1[0], G2W * OUT)], BF16, tag="wT",
                       name=f"w1T_{i}") for i in range(KD)]

        beta1s = pers.tile([P, TT], F32, tag="beta1s")
        beta2s = pers.tile([P, TT], F32, tag="beta2s")
        m2s = pers.tile([P, TT], F32, tag="m2s")
        rb1Ts = pers.tile([1, TT * P], F32, tag="rb1Ts")
        rb2Ts = pers.tile([1, TT * P], F32, tag="rb2Ts")
        beta1_all, rb1T_all, stv2_all, m2_all = [], [], [], []

        # ================= L1 half A =================
        w_fill(w1s_dram, w1T, 0, len(nA) * NC1, SL, range(KD))
        XC = min(512, D)
        for t in range(TT):
            stv = vecs.tile([P, max(1, D // 512), 6], F32, tag="bn_st")
            M1 = vecs.tile([P, 1], F32, tag="M1")
            for cc in range(D // XC):
                xt = xp.tile([P, XC], F32, tag="x")
                nc.sync.dma_start(xt, x_d[t * P:(t + 1) * P,
                                          cc * XC:(cc + 1) * XC])
                nc.vector.bn_stats(stv[:, cc, :], xt)
                mx = vecs.tile([P, 1], F32, tag="mx")
                nc.vector.reduce_max(mx, xt, axis=AX.X,
                                     apply_absolute_value=True)
                if cc == 0:
                    nc.vector.tensor_copy(out=M1, in_=mx)
                else:
                    nc.vector.tensor_tensor(M1, M1, mx, op=ALU.max)
            r1, c1 = finalize_scale(stv, M1, D)
            beta1, rb1T = beta_of(c1, mw1, beta1s[:, t:t + 1],
                                  rb1Ts[0:1, t * P:(t + 1) * P])
            beta1_all.append(beta1)
            rb1T_all.append(rb1T)
            q1 = qp.tile([P, D], BF16, tag="q1")
            for cc in range(D // XC):
                xt = xp.tile([P, XC], F32, tag="x")
                nc.sync.dma_start(xt, x_d[t * P:(t + 1) * P,
                                          cc * XC:(cc + 1) * XC])
                quant_chunk(xt, q1[:, cc * XC:(cc + 1) * XC], r1)
            q1T = qTp.tile([P, KD, P], BF16, tag="q1T")
            q_transpose(q1, q1T, range(KD))
            nc.sync.dma_start(q1T_dram[t * P:(t + 1) * P, :],
                              q1T.rearrange("p a b -> p (a b)"))
            stv2 = pers.tile([P, NCH, 6], F32, tag=f"stv2_{t}")
            m2 = m2s[:, t:t + 1]
            stv2_all.append(stv2)
            m2_all.append(m2)
            w2_prep_piece(2 * P)
            pe(nc.tensor.ldweights(weights=q1T[:, KD - 1, :]))
            for n in nA:
                bc = bch.tile([1, NC1], F32, tag="bc")
                nc.sync.dma_start(bc, b1_d[0:1, n * NC1:(n + 1) * NC1])
                pg = ps_g.tile([P, NC1], F32, tag="ps_g")
                pe(nc.tensor.matmul(pg, rb1T, bc, start=True, stop=False))
                for kd in range(KD):
                    pe(nc.tensor.matmul(pg, q1T[:, kd, :],
                                        w1T[kd][:, n * NC1:(n + 1) * NC1],
                                        start=False, stop=(kd == KD - 1)))
                hc = hcp.tile([P, NC1], F32, tag="hc")
                nc.scalar.activation(hc, pg, AF.Gelu, scale=beta1)
                nc.vector.bn_stats(stv2[:, n, :], hc)
                mx = vecs.tile([P, 1], F32, tag="mx")
                nc.vector.reduce_max(mx, hc, axis=AX.X,
                                     apply_absolute_value=True)
                if n == 0:
                    nc.vector.tensor_copy(out=m2, in_=mx)
                else:
                    nc.vector.tensor_tensor(m2, m2, mx, op=ALU.max)
                nc.sync.dma_start(
                    h_dram[t * P:(t + 1) * P, n * NC1:(n + 1) * NC1], hc)

        w2_prep_piece(OUT)  # any remainder
        mw2 = w_mean(w2sums, w2_chunks_total, OUT * INNER, "mw2")

        # ================= L1 half B + quant =================
        if nB:
            w_fill(w1s_dram, w1T, len(nA) * NC1, len(nB) * NC1, SL, range(KD))
        beta2_all, rb2T_all = [], []
        for t in range(TT):
            if nB:
                q1T = qTp.tile([P, KD, P], BF16, tag="q1T")
                nc.sync.dma_start(
                    q1T.rearrange("p a b -> p (a b)"),
                    q1T_dram[t * P:(t + 1) * P, :])
                pe(nc.tensor.ldweights(weights=q1T[:, KD - 1, :]))
                for n in nB:
                    bc = bch.tile([1, NC1], F32, tag="bc")
                    nc.sync.dma_start(bc, b1_d[0:1, n * NC1:(n + 1) * NC1])
                    pg = ps_g.tile([P, NC1], F32, tag="ps_g")
                    pe(nc.tensor.matmul(pg, rb1T_all[t], bc, start=True,
                                        stop=False))
                    for kd in range(KD):
                        pe(nc.tensor.matmul(
                            pg, q1T[:, kd, :],
                            w1T[kd][:, (n - nB[0]) * NC1:
                                    (n - nB[0] + 1) * NC1],
                            start=False, stop=(kd == KD - 1)))
                    hc = hcp.tile([P, NC1], F32, tag="hc")
                    nc.scalar.activation(hc, pg, AF.Gelu, scale=beta1_all[t])
                    nc.vector.bn_stats(stv2_all[t][:, n, :], hc)
                    mx = vecs.tile([P, 1], F32, tag="mx")
                    nc.vector.reduce_max(mx, hc, axis=AX.X,
                                         apply_absolute_value=True)
                    nc.vector.tensor_tensor(m2_all[t], m2_all[t], mx,
                                            op=ALU.max)
                    nc.sync.dma_start(
                        h_dram[t * P:(t + 1) * P, n * NC1:(n + 1) * NC1], hc)
            r2, c2 = finalize_scale(stv2_all[t], m2_all[t], INNER)
            beta2, rb2T = beta_of(c2, mw2, beta2s[:, t:t + 1],
                                  rb2Ts[0:1, t * P:(t + 1) * P])
            beta2_all.append(beta2)
            rb2T_all.append(rb2T)
            for n in range(NCH):
                hr = hcp.tile([P, NC1], F32, tag="hr")
                nc.sync.dma_start(
                    hr, h_dram[t * P:(t + 1) * P, n * NC1:(n + 1) * NC1])
                q2c = hcp.tile([P, NC1], BF16, tag="q2c")
                quant_chunk(hr, q2c, r2)
                nc.sync.dma_start(
                    q2_dram[t * P:(t + 1) * P, n * NC1:(n + 1) * NC1], q2c)

        # ================= L2 halves =================
        for half in range(2):
            ki0 = half * KIH
            for g in range(KIH // G2W):
                tg = w1T[g]
                for j in range(G2W):
                    ki = ki0 + g * G2W + j
                    nc.sync.dma_start_transpose(
                        tg[:, j * OUT:(j + 1) * OUT], w2s_dram[ki, :, :])
            for t in range(TT):
                q2T = q2Tp.tile([P, KIH, P], BF16, tag="q2T")
                CC2 = min(WCC2, KIH * P)
                for cc in range(KIH * P // CC2):
                    q2r = qp.tile([P, CC2], BF16, tag="q2r")
                    nc.sync.dma_start(
                        q2r, q2_dram[t * P:(t + 1) * P,
                                     ki0 * P + cc * CC2:
                                     ki0 * P + (cc + 1) * CC2])
                    pe(nc.tensor.ldweights(weights=q2r[:, 0:P]))
                    for k in range(CC2 // P):
                        pst = ps_t.tile([P, P], BF16, tag="ps_qt")
                        pe(nc.tensor.transpose(pst, q2r[:, k * P:(k + 1) * P],
                                               identb))
                        nc.scalar.copy(q2T[:, cc * (CC2 // P) + k, :], pst)
                pe(nc.tensor.ldweights(weights=q2T[:, KIH - 1, :]))
                for o in range(OCH):
                    pg = ps_g.tile([P, NC2], F32, tag="ps_g")
                    if half == 0:
                        bc = bch.tile([1, NC2], F32, tag="bc")
                        nc.sync.dma_start(bc, b2_d[0:1, o * NC2:(o + 1) * NC2])
                        pe(nc.tensor.matmul(pg, rb2T_all[t], bc, start=True,
                                            stop=False))
                    for j in range(KIH):
                        pe(nc.tensor.matmul(
                            pg, q2T[:, j, :],
                            w1T[j // G2W][:, (j % G2W) * OUT + o * NC2:
                                          (j % G2W) * OUT + (o + 1) * NC2],
                            start=(half == 1 and j == 0),
                            stop=(j == KIH - 1)))
                    oc = outp.tile([P, NC2], F32, tag="oc")
                    if half == 0:
                        nc.scalar.copy(oc, pg)
                        nc.sync.dma_start(
                            opart_dram[t * P:(t + 1) * P,
                                       o * NC2:(o + 1) * NC2], oc)
                    else:
                        opr = outp.tile([P, NC2], F32, tag="opr")
                        nc.sync.dma_start(
                            opr, opart_dram[t * P:(t + 1) * P,
                                            o * NC2:(o + 1) * NC2])
                        nc.vector.tensor_tensor(oc, opr, pg, op=ALU.add)
                        om = outp.tile([P, NC2], F32, tag="om")
                        nc.scalar.mul(om, oc, beta2_all[t])
                        nc.sync.dma_start(
                            out_d[t * P:(t + 1) * P, o * NC2:(o + 1) * NC2],
                            om)

    nc.compile()
    return nc


@functools.lru_cache(maxsize=1)
def _get_nc():
    return build(TOK, DIM, INNER, DIM)


def kernel(x, w1, b1, w2, b2, _trace=False):
    nc = _get_nc()
    xf = np.ascontiguousarray(x.reshape(B * S, DIM), dtype=np.float32)
    common = {
        "w1": np.ascontiguousarray(w1, dtype=np.float32),
        "b1": np.ascontiguousarray(b1, dtype=np.float32).reshape(1, INNER),
        "w2": np.ascontiguousarray(w2, dtype=np.float32),
        "b2": np.ascontiguousarray(b2, dtype=np.float32).reshape(1, DIM),
        "identf": np.eye(P, dtype=np.float32),
        "identb": np.eye(P, dtype=np.float32).astype(ml_dtypes.bfloat16),
        "ones": np.ones((P, P), dtype=np.float32),
    }
    in_maps = [{"x": xf[c * TOK:(c + 1) * TOK], **common}
               for c in range(NCORES)]
    res = run_bass_kernel_spmd(nc, in_maps, core_ids=list(range(NCORES)),
                               trace=_trace)
    out = np.concatenate([res.results[c]["out"] for c in range(NCORES)],
                         axis=0)
    out = out.reshape(B, S, DIM)
    if _trace:
        return out, res
    return out



# revision 11
# speedup vs baseline: 1.6492x; 1.6492x over previous
"""BitFeedForward (BitNet-style FFN) Trainium2 kernel — 8-core data parallel.

kernel(**inputs) takes the FULL unsharded inputs of
nn_BitFeedForward_25280177504455:
    x  [4, 2048, 2048] f32, w1 [8192, 2048], b1 [8192],
    w2 [2048, 8192], b2 [2048]
and returns the full [4, 2048, 2048] f32 output.

Sharding: data-parallel over tokens (1024 tokens/core).  The host ships
weights pre-transposed (w1.T / w2.T, a pure layout change) so each core
can stream them with large contiguous DMA descriptors, convert to
ternary sign form on the fly (sign -> bf16 in SBUF, no DRAM spill), and
use them directly as matmul operands.  weight_quant's global mean|w| is
computed from per-core shards and combined with a tiny 8-core AllReduce
(8 bytes), so the full |w| reduction is done once across the chip
instead of 8x redundantly.

On-device flow per core (1024 tokens):
  A. |w| partial sums over this core's weight shard -> AllReduce -> mw1, mw2
  B. x stats (rms + absmax) -> r1/c1/beta1, quantize, PE-transpose -> q1T
     (SBUF resident)
  C. L1: stream w1T in 512-col slabs, sign to bf16, matmul
     h[t,i] = gelu(beta1*(q1T.T @ w1s + rb1 x b1)); bn_stats/absmax per
     token accumulate for the second rms; h spilled bf16.
  D. finalize r2/beta2; rebuild q2 from h, PE-transpose -> q2T (SBUF
     resident, aliases q1T's space)
  E. L2: stream w2T in 128-row o-bands, sign, matmul
     out[o,t] = beta2[t]*(w2s.T @ q2T + b2 x rb2) -> out written [o,t];
     host transposes back.
"""
import functools

import numpy as np
import ml_dtypes

from contextlib import ExitStack

import concourse.bacc as bacc
import concourse.tile as tile
from concourse import mybir
from concourse.bass_utils import run_bass_kernel_spmd

F32 = mybir.dt.float32
BF16 = mybir.dt.bfloat16

EPS_RMS = 1e-6
EPS_Q = 1e-5
# v + C lands in [2^23, 2^24) where fp32 spacing is 1.0 -> RNE integer round
C_RND = float(1.5 * 2.0**23)
P = 128
AX = mybir.AxisListType
ALU = mybir.AluOpType
AF = mybir.ActivationFunctionType

NCORES = 8
B, S, DIM = 4, 2048, 2048
INNER = 8192
OUT = DIM
TOK = B * S // NCORES   # 1024 tokens per core
TT = TOK // P           # 8 token tiles
KD = DIM // P           # 16 contraction chunks for L1
KI = INNER // P         # 64 contraction chunks for L2
NE1 = 16                # L1 slabs (512 inner cols each)
SL1 = INNER // NE1      # 512
NB2 = 16                # L2 o-bands (128 out cols each)
BO = OUT // NB2         # 128
WSHE = 2 * INNER * DIM // NCORES  # weight-shard elements per core (4.19M)
WSHC = WSHE // (P * 2048)         # 16 chunks of [128, 2048]


def build():
    from concourse.tile_rust import add_dep_helper

    nc = bacc.Bacc("TRN2", enable_partition_id=False, num_devices=NCORES)

    x_d = nc.dram_tensor("x", [TOK, DIM], F32, kind="ExternalInput")
    w1t_d = nc.dram_tensor("w1t", [DIM, INNER], F32, kind="ExternalInput")
    w2t_d = nc.dram_tensor("w2t", [INNER, OUT], F32, kind="ExternalInput")
    b1_d = nc.dram_tensor("b1", [1, INNER], F32, kind="ExternalInput")
    b2_d = nc.dram_tensor("b2", [1, OUT], F32, kind="ExternalInput")
    wsh_d = nc.dram_tensor("wsh", [P, WSHE // P], F32, kind="ExternalInput")
    idf_d = nc.dram_tensor("identf", [P, P], F32, kind="ExternalInput")
    idb_d = nc.dram_tensor("identb", [P, P], BF16, kind="ExternalInput")
    ones_d = nc.dram_tensor("ones", [P, P], F32, kind="ExternalInput")
    out_d = nc.dram_tensor("out", [OUT, TOK], F32, kind="ExternalOutput")

    with ExitStack() as ctx:
        tc = ctx.enter_context(tile.TileContext(nc))
        pool = lambda name, bufs, space="SBUF": ctx.enter_context(
            tc.tile_pool(name=name, bufs=bufs, space=space))

        consts = pool("consts", 1)
        stag = pool("stag", 3)        # f32 staging: wsh/x/w1T/w2T tiles
        wring = pool("wring", 2)      # bf16 sign-weight slabs
        qTp = pool("qTp", 1)          # q1T then q2T (aliased)
        qp = pool("qp", 2)            # q1 bf16 chunks
        hcp = pool("hcp", 2)          # h / q2 bf16 tiles
        outp = pool("outp", 2)        # f32 drains
        vecs = pool("vecs", 2)
        bch = pool("bch", 2)
        bb = pool("bb", 1)
        pers = pool("pers", 1)
        dram = pool("dram", 1, "DRAM")
        ps_g = pool("ps_g", 4, "PSUM")
        ps_t = pool("ps_t", 2, "PSUM")
        ps_v = pool("ps_v", 2, "PSUM")

        identf = consts.tile([P, P], F32)
        identb = consts.tile([P, P], BF16)
        ones = consts.tile([P, P], F32)
        nc.sync.dma_start(identf, idf_d[:, :])
        nc.sync.dma_start(identb, idb_d[:, :])
        nc.sync.dma_start(ones, ones_d[:, :])

        h_dram = dram.tile([TOK, INNER], BF16)
        cc_in = dram.tile([1, 2], F32)
        cc_out = dram.tile([1, 2], F32, addr_space="Shared")

        state = {"pe": None}

        def pe(instr):
            if state["pe"] is not None:
                add_dep_helper(instr.ins, state["pe"].ins, sync=False,
                               reason="pe chain")
            state["pe"] = instr
            return instr

        # ---- persistent scalars/vectors ----
        wsums4 = pers.tile([P, WSHC * 4], F32, tag="wsums4")
        beta1s = pers.tile([P, TT], F32, tag="beta1s")
        r2s = pers.tile([P, TT], F32, tag="r2s")
        m2s = pers.tile([P, TT], F32, tag="m2s")
        rb1T = pers.tile([1, TOK], F32, tag="rb1T")
        rb2row = pers.tile([1, TOK], F32, tag="rb2row")
        beta2row = pers.tile([1, TOK], F32, tag="beta2row")
        mws = pers.tile([P, 2], F32, tag="mws")
        stv2 = pers.tile([P, TT, NE1, 6], F32, tag="stv2")

        # ================= A: weight-shard |w| sums + AllReduce ========
        for j in range(WSHC * 4):
            wt = stag.tile([P, 512], F32, tag="stag")
            nc.sync.dma_start(wt, wsh_d[:, j * 512:(j + 1) * 512])
            nc.vector.tensor_reduce(wsums4[:, j:j + 1], wt, axis=AX.X,
                                    op=ALU.add, apply_absolute_value=True)
        cc_sb = vecs.tile([1, 2], F32, tag="cc_sb")
        for half in range(2):
            col = vecs.tile([P, 1], F32, tag="wcol")
            nc.vector.tensor_reduce(
                col, wsums4[:, half * (WSHC * 2):(half + 1) * (WSHC * 2)],
                axis=AX.X, op=ALU.add)
            pssc = ps_v.tile([1, 1], F32, tag="psv")
            pe(nc.tensor.matmul(pssc, col, ones[:, 0:1], start=True,
                                stop=True))
            nc.scalar.copy(cc_sb[0:1, half:half + 1], pssc)
        nc.sync.dma_start(cc_in, cc_sb)
        nc.gpsimd.collective_compute(
            "AllReduce", ALU.add,
            replica_groups=[list(range(NCORES))],
            ins=[cc_in.opt()], outs=[cc_out.opt()])
        cc_rb = vecs.tile([1, 2], F32, tag="cc_rb")
        nc.sync.dma_start(cc_rb, cc_out)
        for half in range(2):
            psbc = ps_v.tile([P, 1], F32, tag="psv")
            pe(nc.tensor.matmul(psbc, ones[0:1, :], cc_rb[0:1, half:half + 1],
                                start=True, stop=True))
            nc.scalar.mul(mws[:, half:half + 1], psbc,
                          1.0 / (INNER * DIM * 127.0))
        mw1 = mws[:, 0:1]
        mw2 = mws[:, 1:2]

        def finalize_scale(stv, M, WID, r_out):
            mv = vecs.tile([P, 2], F32, tag="bn_mv")
            nc.vector.bn_aggr(mv, stv)
            msq = vecs.tile([P, 1], F32, tag="msq")
            nc.vector.tensor_tensor(msq, mv[:, 0:1], mv[:, 0:1], op=ALU.mult)
            nc.vector.tensor_tensor(msq, msq, mv[:, 1:2], op=ALU.add)
            nc.vector.tensor_scalar_add(msq, msq, EPS_RMS)
            y = vecs.tile([P, 1], F32, tag="sq_y")
            nc.scalar.sqrt(y, msq)
            d_ = vecs.tile([P, 1], F32, tag="sq_d")
            nc.vector.reciprocal(d_, y)
            nc.vector.tensor_tensor(d_, msq, d_, op=ALU.mult)
            nc.vector.tensor_tensor(y, y, d_, op=ALU.add)
            nc.vector.tensor_scalar_mul(y, y, 0.5 * (float(WID) ** 0.5))
            a = vecs.tile([P, 1], F32, tag="a")
            nc.vector.reciprocal(a, y)
            c = vecs.tile([P, 1], F32, tag="c")
            nc.vector.tensor_tensor(c, a, M, op=ALU.mult)
            nc.vector.tensor_scalar_max(c, c, EPS_Q)
            r = vecs.tile([P, 1], F32, tag="r")
            nc.vector.reciprocal(r, c)
            nc.vector.tensor_tensor(r, r, a, op=ALU.mult)
            nc.vector.tensor_scalar_mul(r_out, r, 127.0)
            return c

        def col_to_row(col, row_slice):
            pst = ps_v.tile([1, P], F32, tag="psv")
            pe(nc.tensor.transpose(pst, col, identf))
            nc.scalar.copy(row_slice, pst)

        # ================= B: x-phase ==================================
        q1T = qTp.tile([P, KD, TOK], BF16, tag="qT", name="q1T")
        for tt in range(TT):
            stv = vecs.tile([P, 4, 6], F32, tag="stv1")
            M1 = vecs.tile([P, 1], F32, tag="M1")
            for cc in range(4):
                xt = stag.tile([P, 512], F32, tag="stag")
                nc.sync.dma_start(xt, x_d[tt * P:(tt + 1) * P,
                                          cc * 512:(cc + 1) * 512])
                nc.vector.bn_stats(stv[:, cc, :], xt)
                mx = vecs.tile([P, 1], F32, tag="mx")
                nc.vector.tensor_reduce(mx, xt, axis=AX.X, op=ALU.max,
                                        apply_absolute_value=True)
                if cc == 0:
                    nc.vector.tensor_copy(out=M1, in_=mx)
                else:
                    nc.vector.tensor_tensor(M1, M1, mx, op=ALU.max)
            r1 = vecs.tile([P, 1], F32, tag="r1")
            c1 = finalize_scale(stv, M1, DIM, r1)
            beta1 = beta1s[:, tt:tt + 1]
            nc.vector.tensor_tensor(beta1, c1, mw1, op=ALU.mult)
            rb1 = vecs.tile([P, 1], F32, tag="rb1")
            nc.vector.reciprocal(rb1, beta1)
            col_to_row(rb1, rb1T[0:1, tt * P:(tt + 1) * P])
            for cc in range(4):
                xt = stag.tile([P, 512], F32, tag="stag")
                nc.sync.dma_start(xt, x_d[tt * P:(tt + 1) * P,
                                          cc * 512:(cc + 1) * 512])
                xq = stag.tile([P, 512], F32, tag="stag")
                nc.scalar.activation(xq, xt, AF.Copy, bias=C_RND, scale=r1)
                q1 = qp.tile([P, 512], BF16, tag="q1")
                nc.vector.tensor_scalar(q1, xq, C_RND, None,
                                        op0=ALU.subtract)
                pst = ps_t.tile([P, 512], BF16, tag="pst")
                for j in range(4):
                    pe(nc.tensor.transpose(pst[:, j * P:(j + 1) * P],
                                           q1[:, j * P:(j + 1) * P], identb))
                nc.scalar.copy(
                    q1T[:, 4 * cc:4 * (cc + 1), tt * P:(tt + 1) * P],
                    pst.rearrange("p (a b) -> p a b", b=P))

        # ================= C: L1 slabs =================================
        for e in range(NE1):
            ws = wring.tile([P, KD, SL1], BF16, tag="w", name=f"ws1_{e}")
            for dc in range(KD):
                wt = stag.tile([P, SL1], F32, tag="stag")
                nc.sync.dma_start(
                    wt, w1t_d[dc * P:(dc + 1) * P, e * SL1:(e + 1) * SL1])
                nc.scalar.sign(ws[:, dc, :], wt)
            bc = bch.tile([1, SL1], F32, tag="bc")
            nc.sync.dma_start(bc, b1_d[0:1, e * SL1:(e + 1) * SL1])
            for tt in range(TT):
                pg = ps_g.tile([P, SL1], F32, tag="psg")
                pe(nc.tensor.matmul(pg, rb1T[0:1, tt * P:(tt + 1) * P], bc,
                                    start=True, stop=False))
                for dc in range(KD):
                    pe(nc.tensor.matmul(pg, q1T[:, dc, tt * P:(tt + 1) * P],
                                        ws[:, dc, :], start=False,
                                        stop=(dc == KD - 1)))
                hc = hcp.tile([P, SL1], BF16, tag="h")
                nc.scalar.activation(hc, pg, AF.Gelu,
                                     scale=beta1s[:, tt:tt + 1])
                nc.vector.bn_stats(stv2[:, tt, e, :], hc)
                mx = vecs.tile([P, 1], F32, tag="mx")
                nc.vector.tensor_reduce(mx, hc, axis=AX.X, op=ALU.max,
                                        apply_absolute_value=True)
                m2 = m2s[:, tt:tt + 1]
                if e == 0:
                    nc.vector.tensor_copy(out=m2, in_=mx)
                else:
                    nc.vector.tensor_tensor(m2, m2, mx, op=ALU.max)
                nc.sync.dma_start(
                    h_dram[tt * P:(tt + 1) * P, e * SL1:(e + 1) * SL1], hc)

        # ================= finalize L2 scales ==========================
        for tt in range(TT):
            r2 = r2s[:, tt:tt + 1]
            c2 = finalize_scale(stv2[:, tt, :, :], m2s[:, tt:tt + 1], INNER,
                                r2)
            beta2 = vecs.tile([P, 1], F32, tag="beta2")
            nc.vector.tensor_tensor(beta2, c2, mw2, op=ALU.mult)
            rb2 = vecs.tile([P, 1], F32, tag="rb2")
            nc.vector.reciprocal(rb2, beta2)
            col_to_row(rb2, rb2row[0:1, tt * P:(tt + 1) * P])
            col_to_row(beta2, beta2row[0:1, tt * P:(tt + 1) * P])
        bb0 = bb.tile([P, 512], F32, tag="bb0")
        bb1 = bb.tile([P, 512], F32, tag="bb1")
        nc.gpsimd.partition_broadcast(bb0, beta2row[0:1, 0:512])
        nc.gpsimd.partition_broadcast(bb1, beta2row[0:1, 512:1024])
        bbs = [bb0, bb1]

        # ================= D: rebuild q2, transpose -> q2T =============
        q2T = qTp.tile([P, KI, TOK], BF16, tag="qT", name="q2T")
        for tt in range(TT):
            for ic in range(INNER // 512):
                hr = hcp.tile([P, 512], BF16, tag="hr")
                nc.sync.dma_start(
                    hr, h_dram[tt * P:(tt + 1) * P, ic * 512:(ic + 1) * 512])
                hq = stag.tile([P, 512], F32, tag="stag")
                nc.scalar.activation(hq, hr, AF.Copy, bias=C_RND,
                                     scale=r2s[:, tt:tt + 1])
                q2c = hcp.tile([P, 512], BF16, tag="q2c")
                nc.vector.tensor_scalar(q2c, hq, C_RND, None,
                                        op0=ALU.subtract)
                pst = ps_t.tile([P, 512], BF16, tag="pst")
                for j in range(4):
                    pe(nc.tensor.transpose(pst[:, j * P:(j + 1) * P],
                                           q2c[:, j * P:(j + 1) * P], identb))
                nc.scalar.copy(
                    q2T[:, 4 * ic:4 * (ic + 1), tt * P:(tt + 1) * P],
                    pst.rearrange("p (a b) -> p a b", b=P))

        # ================= E: L2 o-bands ===============================
        for b in range(NB2):
            ws2 = wring.tile([P, KI, BO], BF16, tag="w", name=f"ws2_{b}")
            for g in range(16):
                wt = stag.tile([P, 4, BO], F32, tag="stag")
                nc.sync.dma_start(
                    wt,
                    w2t_d[g * 512:(g + 1) * 512,
                          b * BO:(b + 1) * BO].rearrange(
                              "(k p) o -> p k o", p=P))
                nc.scalar.sign(ws2[:, 4 * g:4 * (g + 1), :], wt)
            bc2 = bch.tile([1, BO], F32, tag="bc2")
            nc.sync.dma_start(bc2, b2_d[0:1, b * BO:(b + 1) * BO])
            for tg in range(2):
                pb = ps_g.tile([P, 512], F32, tag="psg")
                pe(nc.tensor.matmul(pb, bc2,
                                    rb2row[0:1, tg * 512:(tg + 1) * 512],
                                    start=True, stop=False))
                for kc in range(KI):
                    pe(nc.tensor.matmul(pb, ws2[:, kc, :],
                                        q2T[:, kc, tg * 512:(tg + 1) * 512],
                                        start=False, stop=(kc == KI - 1)))
                ob = outp.tile([P, 512], F32, tag="ob")
                nc.vector.tensor_tensor(ob, pb, bbs[tg], op=ALU.mult)
                nc.sync.dma_start(
                    out_d[b * BO:(b + 1) * BO, tg * 512:(tg + 1) * 512], ob)

    nc.compile()
    return nc


@functools.lru_cache(maxsize=1)
def _get_nc():
    return build()


def kernel(x, w1, b1, w2, b2, _trace=False):
    nc = _get_nc()
    xf = np.ascontiguousarray(x.reshape(B * S, DIM), dtype=np.float32)
    w1 = np.asarray(w1, dtype=np.float32)
    w2 = np.asarray(w2, dtype=np.float32)
    w1f = w1.reshape(-1)
    w2f = w2.reshape(-1)
    shard = w1f.size // NCORES
    common = {
        "w1t": np.ascontiguousarray(w1.T),
        "w2t": np.ascontiguousarray(w2.T),
        "b1": np.ascontiguousarray(b1, dtype=np.float32).reshape(1, INNER),
        "b2": np.ascontiguousarray(b2, dtype=np.float32).reshape(1, OUT),
        "identf": np.eye(P, dtype=np.float32),
        "identb": np.eye(P, dtype=np.float32).astype(ml_dtypes.bfloat16),
        "ones": np.ones((P, P), dtype=np.float32),
    }
    in_maps = []
    for c in range(NCORES):
        wsh = np.concatenate([
            w1f[c * shard:(c + 1) * shard].reshape(P, -1),
            w2f[c * shard:(c + 1) * shard].reshape(P, -1)], axis=1)
        in_maps.append({
            "x": xf[c * TOK:(c + 1) * TOK],
            "wsh": np.ascontiguousarray(wsh),
            **common,
        })
    res = run_bass_kernel_spmd(nc, in_maps, core_ids=list(range(NCORES)),
                               trace=_trace)
    out = np.concatenate(
        [res.results[c]["out"].T for c in range(NCORES)], axis=0)
    out = out.reshape(B, S, DIM)
    if _trace:
        return out, res
    return out
